# revision 30
# baseline (speedup 1.0000x reference)
"""Trainium2 Bass kernel for HCEN forward.

The reference is fully linear:
  out = (x.mean(1) @ W_enc.T + b_enc) @ W_out.T + b_out
      = x.mean(1) @ (W_out@W_enc).T + (W_out@b_enc + b_out)
so the two layers fold into ONE fused weight (host matmul), halving weight
traffic and removing the layer1->transpose->layer2 tail.

Sharding: data-parallel over batch, B=16 across 8 cores -> 2 batches/core.

d-split ownership of the seq-mean (no seq-split combine): the PE owns
d-chunks 0..2 end-to-end (fp8 ones-matmul, DoubleRow: 2 seq rows/cell-cycle),
ACT/DVE/gpsimd own d-chunks 3..7 (int8 free-axis reductions; gpsimd folds
i8+i8->bf16 exactly, the fold's final reduce ("tax") lands on ACT or DVE).
Each output d-chunk is finalized by exactly one path, so the fused layer
matmul accumulates into PSUM chunk-by-chunk as each mean column finalizes.
The bias enters PSUM via an early rank-1 fp32 matmul (ones[1,2] x bf[1,O]),
so the tail is: last quarter-chunk reduce -> mt -> 2 matmuls -> per-bank
PSUM->SBUF copies (ACT n0 / DVE n1) -> out DMA.

HBM traffic/core ~9.5 MB: x 8.4 MB at 1 B/elem, fused weight 1 MB (e3m4
scaled into +-8; descale folded into the mean scales; bf16 stationary x
e3m4 moving matmul verified on HW), small consts. The DMA stream primes
the int8 lanes first, interleaves xpe pieces so the PE never backlogs,
and delivers the last two chunks as halves/quarters spread across lanes.
Pool bufs cover every in-flight piece (no head-of-line DMA blocking).
"""

import os
import sys
from contextlib import ExitStack

import ml_dtypes
import numpy as np

for _p in ("/opt/trn_rl_repo", "/root/.axon_site/_ro/trn_rl_repo"):
    if os.path.isdir(_p) and _p not in sys.path:
        sys.path.insert(0, _p)

import concourse.bass as bass  # noqa: E402
import concourse.tile as tile  # noqa: E402
from concourse import bacc, mybir  # noqa: E402
from concourse.bass_utils import run_bass_kernel_spmd  # noqa: E402

B, S, D, O = 16, 4096, 1024, 1024
NCORES = 8
BPC = B // NCORES
P = 128
K = 3  # PE-owned d-chunks
DPE = K * P  # 384
NC8 = 8 - K  # int8 d-chunks (global chunks 3..7)
D8 = NC8 * P  # 640
NF = 512
QTOT = S // P  # 32
XQ = [(0, 10), (10, 20), (20, 28), (28, 32)]  # xpe piece q-ranges
XPAIRS = [range(0, 5), range(5, 10), range(10, 14), range(14, 16)]

F32 = mybir.dt.float32
BF16 = mybir.dt.bfloat16
FP8 = mybir.dt.float8e4
FP8W = mybir.dt.float8e3
I8 = mybir.dt.int8
DR = mybir.MatmulPerfMode.DoubleRow
COPY = mybir.ActivationFunctionType.Copy

# int8 pieces: (batch, local chunk 0..4, s_lo, s_hi, lane, parts col)
# lanes: A=ACT reduce, V=DVE reduce, Ga/Gv=gps fold with tax on ACT/DVE
PIECES = {
    "c3b0": (0, 0, 0, 4096, "V", 0),
    "c3b1": (1, 0, 0, 4096, "Gv", 1),
    "c4b0": (0, 1, 0, 4096, "Gv", 2),
    "c4b1": (1, 1, 0, 4096, "A", 3),
    "c5b0": (0, 2, 0, 4096, "Ga", 4),
    "c5b1": (1, 2, 0, 4096, "V", 5),
    "c6b0h1": (0, 3, 0, 2048, "A", 6),
    "c6b0h2": (0, 3, 2048, 4096, "V", 7),
    "c6b1h1": (1, 3, 0, 2048, "Ga", 8),
    "c6b1h2": (1, 3, 2048, 4096, "V", 9),
    "c7b0h1": (0, 4, 0, 2048, "A", 10),
    "c7b0h2": (0, 4, 2048, 4096, "Ga", 11),
    "c7b1q1": (1, 4, 0, 1024, "V", 12),
    "c7b1q2": (1, 4, 1024, 2048, "V", 13),
    "c7b1q3": (1, 4, 2048, 3072, "Ga", 14),
    "c7b1q4": (1, 4, 3072, 4096, "Gv", 15),
}


def build_nc():
    nc = bacc.Bacc(
        "TRN2",
        target_bir_lowering=False,
        debug=False,
        enable_asserts=False,
        num_devices=NCORES,
    )
    xpe_ext = nc.dram_tensor("xpe", [BPC, S, DPE], FP8, kind="ExternalInput").ap()
    x8_ext = nc.dram_tensor("x8", [BPC, D8, S], I8, kind="ExternalInput").ap()
    wf_ext = nc.dram_tensor("wf", [P, NC8 + K, O], FP8W, kind="ExternalInput").ap()
    scl_ext = nc.dram_tensor("scl", [2], F32, kind="ExternalInput").ap()
    bf_ext = nc.dram_tensor("bf", [O], F32, kind="ExternalInput").ap()
    out_ext = nc.dram_tensor("out", [BPC, O], F32, kind="ExternalOutput").ap()

    with ExitStack() as ctx:
        tc = ctx.enter_context(tile.TileContext(nc))
        consts = ctx.enter_context(tc.tile_pool(name="consts", bufs=1))
        wpool = ctx.enter_context(tc.tile_pool(name="wpool", bufs=1))
        xfull = ctx.enter_context(tc.tile_pool(name="xfull", bufs=6))
        xtail = ctx.enter_context(tc.tile_pool(name="xtail", bufs=16))
        gpool = ctx.enter_context(tc.tile_pool(name="gpool", bufs=9))
        spool = ctx.enter_context(tc.tile_pool(name="spool", bufs=1))
        ppe = ctx.enter_context(tc.tile_pool(name="ppe", bufs=2, space="PSUM"))
        pp2 = ctx.enter_context(tc.tile_pool(name="pp2", bufs=1, space="PSUM"))

        # ---- consts / warmup ----
        ones_dr = consts.tile([P, 2, 16], FP8)  # DR stationary: 16 B Ko stride
        nc.gpsimd.memset(ones_dr[:], 1.0)
        ident1 = consts.tile([1, 1], F32)
        nc.gpsimd.memset(ident1[:], 1.0)
        ones2f = consts.tile([1, 2], F32)
        nc.gpsimd.memset(ones2f[:], 1.0)
        parts = spool.tile([P, 26], F32, name="parts")
        nc.gpsimd.memset(parts[:], 0.0)
        actwarm = spool.tile([1, 1], F32, name="actwarm")
        nc.scalar.copy(actwarm[:], ident1[:])  # pre-load ACT table

        mt_bf = spool.tile([P, 8, BPC], BF16, name="mt_bf")
        pe_sb = [spool.tile([1, DPE], F32, name=f"pe_sb{b}") for b in range(BPC)]
        out_sb = spool.tile([BPC, O], F32, name="out_sb")
        scales_bc = consts.tile([P, 2], F32, name="scales_bc")
        bf_row = consts.tile([1, O], F32, name="bf_row")
        wf_sb = wpool.tile([P, 8, O], FP8W)
        g_act = spool.tile([P, S], I8, name="g_act")  # ACT copy sink (i8)
        g_actb = spool.tile([P, S // 2], BF16, name="g_actb")  # ACT tax sink
        s4 = spool.tile([P, 4], F32, name="s4")  # ACT 4-col combine sink
        tp_ps = pp2.tile([P, 2 * K], F32, name="tp_ps", tag="tp")
        out_ps = pp2.tile([BPC, O], F32, name="out_ps", tag="ops")
        pe_ps = [ppe.tile([1, DPE], F32, name=f"pe_ps{b}", tag=f"pe{b}")
                 for b in range(BPC)]
        xpe_sb = [spool.tile([P, QTOT, DPE], FP8, name=f"xpe_sb{b}")
                  for b in range(BPC)]

        xt_tiles = {}
        g16_tiles = {}

        def dma_piece(key, q=None):
            b, lc, slo, shi, _, _ = PIECES[key]
            n = shi - slo
            if n < S:
                t = xtail.tile([P, S // 2], I8, name=f"xt_{key}", tag="xt")
            else:
                t = xfull.tile([P, S], I8, name=f"xt_{key}", tag="xf")
            xt_tiles[key] = t[:, 0:n]
            (q or nc.sync).dma_start(
                t[:, 0:n], x8_ext[b, lc * P:(lc + 1) * P, slo:shi])

        def dma_xpe(b, g):
            qlo, qhi = XQ[g]
            nc.sync.dma_start(
                xpe_sb[b][:, qlo:qhi, :],
                xpe_ext[b, :, :].rearrange("(p q) d -> p q d", p=P)[:, qlo:qhi, :],
            )

        def fold(key):
            """gpsimd i8+i8->bf16 fold (first half of a team reduce)."""
            _, _, slo, shi, _, _ = PIECES[key]
            n2 = (shi - slo) // 2
            g16 = gpool.tile([P, S // 2], BF16, name=f"g16_{key}", tag="g16")
            g16_tiles[key] = g16[:, 0:n2]
            xt = xt_tiles[key]
            nc.gpsimd.tensor_add(g16[:, 0:n2], xt[:, 0:n2], xt[:, n2:2 * n2])

        def red(key):
            """direct reduce (lane A or V) of an int8 piece."""
            _, _, _, _, lane, col = PIECES[key]
            xt = xt_tiles[key]
            if lane == "A":
                nc.scalar.activation(g_act[:, 0:xt.shape[-1]], xt, COPY,
                                     accum_out=parts[:, col:col + 1])
            else:
                nc.vector.tensor_reduce(parts[:, col:col + 1], xt,
                                        op=mybir.AluOpType.add,
                                        axis=mybir.AxisListType.X)

        def tax(key):
            """second half of a team reduce (bf16 -> parts col)."""
            _, _, _, _, lane, col = PIECES[key]
            g16 = g16_tiles[key]
            if lane == "Ga":
                nc.scalar.activation(g_actb[:, 0:g16.shape[-1]], g16, COPY,
                                     accum_out=parts[:, col:col + 1])
            else:
                nc.vector.tensor_reduce(parts[:, col:col + 1], g16,
                                        op=mybir.AluOpType.add,
                                        axis=mybir.AxisListType.X)

        def mt_one(eng, gc, b, col):
            e = nc.scalar if eng == "A" else nc.vector
            if eng == "A":
                e.mul(mt_bf[:, gc, b:b + 1], parts[:, col:col + 1],
                      scales_bc[:, 0:1])
            else:
                nc.vector.tensor_scalar_mul(mt_bf[:, gc, b:b + 1],
                                            parts[:, col:col + 1],
                                            scales_bc[:, 0:1])

        def mt_pair(eng, gc, b, cola, colb, tmpcol):
            nc.vector.tensor_add(parts[:, tmpcol:tmpcol + 1],
                                 parts[:, cola:cola + 1],
                                 parts[:, colb:colb + 1])
            mt_one(eng, gc, b, tmpcol)

        def mt_quad(eng, gc, b, col0, tmpcol):
            # combine 4 quarter cols then scale, all on eng
            if eng == "V":
                nc.vector.tensor_reduce(parts[:, tmpcol:tmpcol + 1],
                                        parts[:, col0:col0 + 4],
                                        op=mybir.AluOpType.add,
                                        axis=mybir.AxisListType.X)
            else:
                nc.scalar.activation(s4[:], parts[:, col0:col0 + 4], COPY,
                                     accum_out=parts[:, tmpcol:tmpcol + 1])
            mt_one(eng, gc, b, tmpcol)

        def pe_pairs(g):
            for j in XPAIRS[g]:
                for b in range(BPC):
                    nc.tensor.matmul(
                        pe_ps[b][:], ones_dr[:, :, 0:1],
                        xpe_sb[b][:, 2 * j:2 * j + 2, :],
                        start=(j == 0), stop=(j == 15), perf_mode=DR)

        def layer(gc, start=False, stop=False):
            for n in range(O // NF):
                nc.tensor.matmul(
                    out_ps[:, n * NF:(n + 1) * NF],
                    mt_bf[:, gc, :],
                    wf_sb[:, gc, n * NF:(n + 1) * NF],
                    start=start, stop=stop)

        # ================= stream schedule =================
        dma_piece("c3b0"); red("c3b0")
        dma_piece("c3b1"); fold("c3b1")
        nc.sync.dma_start(scales_bc[:], scl_ext[None, :].broadcast_to([P, 2]))
        nc.sync.dma_start(bf_row[:], bf_ext[None, :])
        dma_piece("c4b0"); fold("c4b0")
        # bias enters PSUM first (rank-1 fp32 matmul), so it is never tail work
        for n in range(O // NF):
            nc.tensor.matmul(out_ps[:, n * NF:(n + 1) * NF], ones2f[:],
                             bf_row[:, n * NF:(n + 1) * NF],
                             start=True, stop=False)
        mt_one("V", 3, 0, 0)
        tax("c3b1"); mt_one("V", 3, 1, 1)
        dma_xpe(0, 0); dma_xpe(1, 0)
        pe_pairs(0)
        dma_piece("c4b1"); red("c4b1"); mt_one("A", 4, 1, 3)
        dma_piece("c5b0"); fold("c5b0")
        tax("c4b0"); mt_one("V", 4, 0, 2)
        dma_xpe(0, 1); dma_xpe(1, 1)
        pe_pairs(1)
        dma_piece("c5b1"); red("c5b1"); mt_one("V", 5, 1, 5)
        tax("c5b0"); mt_one("A", 5, 0, 4)
        nc.sync.dma_start(wf_sb[:], wf_ext[:, :, :])
        dma_xpe(0, 2); dma_xpe(1, 2)
        pe_pairs(2)
        dma_xpe(0, 3); dma_xpe(1, 3)
        pe_pairs(3)
        # PE partials -> partition layout (copies split A/V, mid-stream)
        nc.scalar.copy(pe_sb[0][:], pe_ps[0][:])
        nc.vector.tensor_copy(pe_sb[1][:], pe_ps[1][:])
        for c in range(K):
            for b in range(BPC):
                nc.tensor.transpose(
                    tp_ps[:, 2 * c + b:2 * c + b + 1],
                    pe_sb[b][:, c * P:(c + 1) * P], ident1[:])
        nc.scalar.mul(mt_bf[:, 0:K, :].rearrange("p c b -> p (c b)"),
                      tp_ps[:], scales_bc[:, 1:2])
        layer(3)
        layer(4)
        layer(5)
        for c in range(K):
            layer(c)

        # ---- int8 tail ----
        dma_piece("c6b0h1"); red("c6b0h1")
        dma_piece("c6b0h2"); red("c6b0h2")
        mt_pair("V", 6, 0, 6, 7, 22)
        dma_piece("c6b1h1"); fold("c6b1h1")
        dma_piece("c6b1h2"); red("c6b1h2")
        dma_piece("c7b0h1"); red("c7b0h1")
        dma_piece("c7b0h2"); fold("c7b0h2")
        tax("c6b1h1"); mt_pair("A", 6, 1, 8, 9, 23)
        layer(6)
        dma_piece("c7b1q1"); red("c7b1q1")
        dma_piece("c7b1q2"); red("c7b1q2")
        dma_piece("c7b1q3"); fold("c7b1q3")
        dma_piece("c7b1q4"); fold("c7b1q4")
        tax("c7b0h2"); mt_pair("A", 7, 0, 10, 11, 24)
        tax("c7b1q4")
        tax("c7b1q3")
        nc.vector.tensor_reduce(parts[:, 25:26], parts[:, 12:16],
                                op=mybir.AluOpType.add,
                                axis=mybir.AxisListType.X)
        mt_one("V", 7, 1, 25)
        layer(7, stop=True)

        # per-bank PSUM -> SBUF -> HBM, two parallel paths
        nc.scalar.copy(out_sb[:, 0:NF], out_ps[:, 0:NF])
        nc.vector.tensor_copy(out_sb[:, NF:O], out_ps[:, NF:O])
        nc.scalar.dma_start(out_ext[:, 0:NF], out_sb[:, 0:NF])
        nc.sync.dma_start(out_ext[:, NF:O], out_sb[:, NF:O])

    nc.compile()
    return nc


_CACHE = {}


def _cached_nc():
    if "nc" not in _CACHE:
        _CACHE["nc"] = build_nc()
    return _CACHE["nc"]


def make_in_maps(x, W_enc, b_enc, W_out, b_out):
    x = np.asarray(x, dtype=np.float32)
    W_enc = np.asarray(W_enc, dtype=np.float32)
    b_enc = np.asarray(b_enc, dtype=np.float32)
    W_out = np.asarray(W_out, dtype=np.float32)
    b_out = np.asarray(b_out, dtype=np.float32)

    Wf = (W_out.astype(np.float64) @ W_enc.astype(np.float64)).astype(np.float32)
    bfu = (W_out.astype(np.float64) @ b_enc.astype(np.float64) + b_out).astype(
        np.float32)

    qs = float(np.abs(x).max()) / 127.0
    sw = 8.0 / float(np.abs(Wf).max())  # e3m4 headroom (max normal 15.5)

    xpe = np.ascontiguousarray(x[:, :, :DPE]).astype(ml_dtypes.float8_e4m3fn)
    x8 = np.ascontiguousarray(
        np.rint(x[:, :, DPE:] * (1.0 / qs)).astype(np.int8).transpose(0, 2, 1))
    # [P, 8, O]: one contiguous 8 KB line per partition (cheap DMA dispatch)
    wf8 = np.ascontiguousarray(
        (Wf.T * sw).astype(ml_dtypes.float8_e3m4)
        .reshape(8, P, O).transpose(1, 0, 2))
    scl = np.array([qs / (S * sw), 1.0 / (S * sw)], dtype=np.float32)
    return [
        {
            "xpe": xpe[i * BPC:(i + 1) * BPC],
            "x8": x8[i * BPC:(i + 1) * BPC],
            "wf": wf8,
            "scl": scl,
            "bf": bfu,
        }
        for i in range(NCORES)
    ]


def gather_out(results):
    return np.ascontiguousarray(
        np.concatenate([results[i]["out"] for i in range(NCORES)], axis=0))


def kernel(x, W_enc, b_enc, W_out, b_out):
    nc = _cached_nc()
    in_maps = make_in_maps(x, W_enc, b_enc, W_out, b_out)
    res = run_bass_kernel_spmd(nc, in_maps, list(range(NCORES)))
    return gather_out(res.results)
